# revision 42
# baseline (speedup 1.0000x reference)
"""Trainium2 Bass kernel for nn_MultiHeadAttention_764504179141.

Reference computation (per batch element n; S=2048, D=1024):
    scaled    = q * k / sqrt(D)                    (N,S,D) elementwise
    alignment = scaled @ W_align.T + b_align       (N,S,D)
    attn      = softmax(alignment, axis=1)         softmax over S
    out       = (attn * value.sum(-1, keepdims)) @ W_ctx.T + b_ctx

Sharding: pure data parallel — batch element n -> NeuronCore n (N == 8 cores).
Weights are replicated; the host pre-transposes/casts/scales them once.

Per-core dataflow (both big matmuls run as fp8e4m3 DoubleRow, which packs two
K-tiles per pass; fp32 psum accumulation throughout):
    1. load packed q|k s-tiles (fp32), DVE multiply -> scaled (bf16, s-major)
    2. PE transpose scaled -> scaledT [d,s] (the PE contracts over the
       partition dim), casting to fp8 during the psum->sbuf evacuation
    3. mm1: AT[e,s] = WaT.T @ scaledT (fp8 DR), accumulated over d in PSUM
    4. ACT exp(AT/(32*sqrt D)) -> P fp32 tmp, accum_out = softmax denom
       (softmax over s = free axis; no max-subtraction needed: |arg| < 0.2
       for these inputs; b_align cancels in the softmax and is dropped)
    5. x = 32*(P-1) stored fp8 [e,s]; WcT scaled by 65536/denom[e] into fp8
       (centering: P ~= 1, so mm2 runs on the small residual and the bulk
       term comes from host-exact column sums)
    6. mm2: psum[s,f] seeded with colsum via K=1 fp16 matmul (ones x wcs),
       then x.T @ WcT' accumulates (fp8 DR) over e
    7. ACT copy psum -> sbuf scaled by vsum[s]/(32*65536) per partition
       (vsum = value row sums, computed by DVE reduce), DMA out.
"""

import numpy as np
import ml_dtypes

import concourse.bacc as bacc
import concourse.bass as bass
import concourse.mybir as mybir
import concourse.tile as tile
from concourse import masks
from concourse.bass_utils import run_bass_kernel_spmd

N, S, D = 8, 2048, 1024
P = 128
NS = S // P   # 16 s-tiles
ND = D // P   # 8 d/e/f-tiles
NCORES = 8
FP32 = mybir.dt.float32
BF16 = mybir.dt.bfloat16
FP8 = mybir.dt.float8e4
FP8_NP = mybir.dt.np(FP8)
# W_align.T is pre-scaled by WA_PRESCALE on the host so its entries (std
# ~1/32) land in fp8e4m3's normal range; the exp() activation scale divides
# it back out together with the 1/sqrt(D) attention scale.
WA_PRESCALE = 32.0
SCALE = float(1.0 / np.sqrt(D) / WA_PRESCALE)
# mm2 runs on centered attention weights: P = exp(...) ~= 1, so we compute
# out = vsum * (colsum + (P-1).T @ WcT/denom). x = 32*(P-1) is stored fp8
# (std ~1), WcT/denom is scaled by WC_PRESCALE into fp8's normal range, and
# the psum is pre-initialized with colsum (host-computed column sums of
# W_ctx, scaled to match). The evacuation divides everything back out.
X_PRESCALE = 32.0
WC_PRESCALE = 65536.0
EVAC_SCALE = 1.0 / (X_PRESCALE * WC_PRESCALE)


def build(n_iters=1):
    nc = bacc.Bacc("TRN2", target_bir_lowering=False, debug=False)
    # q and k packed as one tensor [2S, D] (q rows then k rows) so each
    # (q_tile, k_tile) pair arrives via ONE dma_start — the multiply that
    # consumes both then needs only one sync-wait (the TT instruction
    # encoding has room for a single wait command).
    qk = nc.dram_tensor("qk", [2 * S, D], FP32, kind="ExternalInput").ap()
    v = nc.dram_tensor("v", [S, D], FP32, kind="ExternalInput").ap()
    # W_align.T laid out (d, e) row-major, fp8e4m3 pre-scaled (host-prepped)
    wat = nc.dram_tensor("wat", [D, D], FP8, kind="ExternalInput").ap()
    # W_ctx.T laid out (e, f) row-major, bf16 (host-prepped)
    wct = nc.dram_tensor("wct", [D, D], BF16, kind="ExternalInput").ap()
    # host-computed column sums of WcT, pre-scaled: X*WC/2048 * sum_e WcT[e,f].
    # fp16 (not bf16): it seeds the mm2 psum via a K=1 matmul, and its ~1e-4
    # relative rounding on this (dominant) term stays far below the fp8 noise.
    wcs = nc.dram_tensor("wcs", [1, D], mybir.dt.float16, kind="ExternalInput").ap()
    out = nc.dram_tensor("out", [S, D], FP32, kind="ExternalOutput").ap()
    qk_r = qk.rearrange("(h s) d -> s h d", h=2)  # [s, {q,k}, d]

    with tile.TileContext(nc) as tc:
        with (
            tc.tile_pool(name="const", bufs=1) as constp,
            tc.tile_pool(name="big", bufs=1) as bigp,
            tc.tile_pool(name="io", bufs=3) as iop,
            tc.tile_pool(name="stp", bufs=3) as stp,
            tc.tile_pool(name="pst", bufs=2, space="PSUM") as psum_t,
            tc.tile_pool(name="ps1", bufs=2, space="PSUM") as psum_1,
            tc.tile_pool(name="ps2", bufs=3, space="PSUM") as psum_2,
        ):
            ident = constp.tile([P, P], BF16)
            masks.make_identity(nc, ident)
            wat_sb = constp.tile([P, ND, D], FP8)    # [p, j, e]; d = j*128+p
            wct_sb = constp.tile([P, ND, D], BF16)   # [p, m, f]; e = m*128+p

            ones_h = constp.tile([1, P], mybir.dt.float16)
            nc.vector.memset(ones_h, 1.0)
            wcs_sb = constp.tile([1, D], mybir.dt.float16)
            nc.scalar.dma_start(wcs_sb, wcs)

            body(nc, tc, n_iters, qk_r, v, wat, wct, out,
                 constp, bigp, iop, stp, psum_t, psum_1, psum_2,
                 ident, wat_sb, wct_sb, ones_h, wcs_sb)

    nc.compile()
    return nc


def body(nc, tc, n_iters, qk_r, v, wat, wct, out,
         constp, bigp, iop, stp, psum_t, psum_1, psum_2,
         ident, wat_sb, wct_sb, ones_h, wcs_sb):
    def one_iter():
            scaledT = bigp.tile([P, ND, S], FP8, name="scaledT")  # [p, j, s]
            pmat = bigp.tile([P, ND, S], FP8, name="pmat")  # x=32*(P-1)
            vsum = bigp.tile([P, NS], FP32, name="vsum")    # [p, i]
            vsum_c = bigp.tile([P, NS], FP32, name="vsum_c")
            dparts = bigp.tile([P, ND, 4], FP32, name="dparts")
            dfull = bigp.tile([P, ND], FP32, name="dfull")
            denb = bigp.tile([P, ND], FP32, name="denb")
            dinv = bigp.tile([P, ND], FP32, name="dinv")
            wct_s = bigp.tile([P, ND, D], FP8, name="wct_s")

            # Phase A+B interleaved by 512-wide s-chunk so the PE alternates
            # transposes and mm1 without long stalls on input DMA.
            for sc in range(4):
                for i in range(4 * sc, 4 * sc + 4):
                    qkt = iop.tile([P, 2, D], FP32, tag="qkt")
                    nc.sync.dma_start(qkt, qk_r[i * P:(i + 1) * P])
                    if sc == 0 and i == 0:
                        # queued behind the first q/k tile so the PE's first
                        # transposes aren't stuck behind 2MB of weights, but
                        # early enough to beat the first mm1.
                        nc.sync.dma_start(
                            wat_sb, wat.rearrange("(j p) e -> p j e", p=P)
                        )
                    st = stp.tile([P, D], BF16, tag="st")
                    nc.vector.tensor_mul(st, qkt[:, 0, :], qkt[:, 1, :])
                    tp = psum_t.tile([P, ND * P], BF16, tag="tp")
                    for j in range(ND):
                        nc.tensor.transpose(
                            tp[:, j * P:(j + 1) * P], st[:, j * P:(j + 1) * P], ident
                        )
                    nc.vector.tensor_copy(
                        scaledT[:, :, i * P:(i + 1) * P],
                        tp.rearrange("p (j c) -> p j c", j=ND),
                    )
                for m in range(ND):
                    at = psum_1.tile([P, 512], FP32, tag="at")
                    for jj in range(ND // 2):
                        # fp8 DoubleRow: contracts over two d-tiles at once
                        # (the paired slices along dim 1).
                        nc.tensor.matmul(
                            at,
                            wat_sb[:, 2 * jj:2 * jj + 2, m * P:(m + 1) * P],
                            scaledT[:, 2 * jj:2 * jj + 2, sc * 512:(sc + 1) * 512],
                            start=(jj == 0),
                            stop=(jj == ND // 2 - 1),
                            perf_mode=mybir.MatmulPerfMode.DoubleRow,
                        )
                    ptmp = stp.tile([P, 512], FP32, tag="ptmp")
                    nc.scalar.activation(
                        ptmp,
                        at,
                        mybir.ActivationFunctionType.Exp,
                        scale=SCALE,
                        accum_out=dparts[:, m, sc:sc + 1],
                    )
                    nc.vector.tensor_scalar(
                        pmat[:, m, sc * 512:(sc + 1) * 512],
                        ptmp,
                        X_PRESCALE,
                        X_PRESCALE,
                        mybir.AluOpType.mult,
                        mybir.AluOpType.subtract,
                    )

            # W_ctx.T arrives on the second HWDGE ring (scalar engine) so it
            # doesn't delay the v loads queued on the sync ring.
            nc.scalar.dma_start(wct_sb, wct.rearrange("(m p) f -> p m f", p=P))

            # softmax denominators -> fold into WcT. Emitted before the vsum
            # reduces so the DVE unblocks mm2 as soon as the last exp lands.
            for m in range(ND):
                nc.vector.reduce_sum(
                    dfull[:, m:m + 1], dparts[:, m, :], axis=mybir.AxisListType.X
                )
            nc.vector.tensor_scalar_mul(denb, dfull, 1.0 / WC_PRESCALE)
            nc.vector.reciprocal(dinv, denb)
            for m in range(ND):
                nc.vector.tensor_scalar_mul(
                    wct_s[:, m, :], wct_sb[:, m, :], dinv[:, m:m + 1]
                )

            # value row sums (only needed for the output stage)
            for i in range(NS):
                vt = iop.tile([P, D], FP32, tag="vt")
                nc.sync.dma_start(vt, v[i * P:(i + 1) * P, :])
                nc.vector.reduce_sum(vsum[:, i:i + 1], vt, axis=mybir.AxisListType.X)
                nc.vector.tensor_scalar_mul(
                    vsum_c[:, i:i + 1], vsum[:, i:i + 1], EVAC_SCALE
                )

            # mm2 (fp8 DoubleRow on centered P) + vsum scale + store. The psum
            # is seeded with the colsum term by a K=1 fp16 matmul (ones col x
            # colsum row), then the x.T @ WcT' matmuls accumulate on top.
            for i in range(NS):
                ot = iop.tile([P, D], FP32, tag="ot")
                for fc in range(2):
                    o2 = psum_2.tile([P, 512], FP32, tag="o2")
                    nc.tensor.matmul(
                        o2,
                        ones_h,
                        wcs_sb[:, fc * 512:(fc + 1) * 512],
                        start=True,
                        stop=False,
                    )
                    for jj in range(ND // 2):
                        nc.tensor.matmul(
                            o2,
                            pmat[:, 2 * jj:2 * jj + 2, i * P:(i + 1) * P],
                            wct_s[:, 2 * jj:2 * jj + 2, fc * 512:(fc + 1) * 512],
                            start=False,
                            stop=(jj == ND // 2 - 1),
                            perf_mode=mybir.MatmulPerfMode.DoubleRow,
                        )
                    nc.scalar.activation(
                        ot[:, fc * 512:(fc + 1) * 512],
                        o2,
                        mybir.ActivationFunctionType.Copy,
                        scale=vsum_c[:, i:i + 1],
                    )
                nc.sync.dma_start(out[i * P:(i + 1) * P, :], ot)

    if n_iters == 1:
        one_iter()
    else:
        # benchmarking mode: run the body n_iters times inside one NEFF so
        # per-iteration time can be measured as a wall-clock difference.
        with tc.For_i(0, n_iters, 1):
            one_iter()


_CACHE = {}


def _built():
    if "nc" not in _CACHE:
        _CACHE["nc"] = build()
    return _CACHE["nc"]


def kernel(query, key, value, W_align, b_align, W_ctx, b_ctx):
    # b_align is intentionally unused: it is constant along the softmax axis
    # (dim 1 / S), so it cancels exactly inside the softmax.
    query = np.asarray(query, np.float32)
    key = np.asarray(key, np.float32)
    value = np.asarray(value, np.float32)
    qk = np.concatenate([query, key], axis=1)  # (N, 2S, D)
    wat = np.ascontiguousarray(
        np.asarray(W_align, np.float32).T * np.float32(WA_PRESCALE)
    ).astype(FP8_NP)
    wct = np.ascontiguousarray(np.asarray(W_ctx, np.float32).T).astype(
        ml_dtypes.bfloat16
    )
    wcs = (
        np.asarray(W_ctx, np.float64).sum(axis=1)
        * (X_PRESCALE * WC_PRESCALE / 2048.0)
    ).astype(np.float16).reshape(1, D)

    nc = _built()
    in_maps = [
        {
            "qk": np.ascontiguousarray(qk[n]),
            "v": np.ascontiguousarray(value[n]),
            "wat": wat,
            "wct": wct,
            "wcs": wcs,
        }
        for n in range(NCORES)
    ]
    res = run_bass_kernel_spmd(nc, in_maps, list(range(NCORES)))
    out = np.stack([res.results[n]["out"] for n in range(NCORES)], axis=0)
    b_ctx = np.asarray(b_ctx, np.float32)
    if b_ctx.any():
        out = out + b_ctx[None, None, :]
    return out.astype(np.float32, copy=False)
